# revision 11
# baseline (speedup 1.0000x reference)
"""Trainium2 Bass kernel for nn_DensityRatioEstimator (InfoNCE-style Cauchy-kernel loss).

Math: logits[i,j] = -log(1 + ||z_y_i - z_x_j||^2). All six outputs are scalar
reductions of the 8192x8192 logit matrix. Key identities used on device:
    exp(logit)     = 1/(1+d2)  = r      (logsumexp needs no max-subtraction: r <= 1)
    sigmoid(logit) = 1/(2+d2)  = r/(1+r) ~= r - r^2 + r^3 ...
Per core the slab work per [128, 4096] chunk is: one K=128 fp32 matmul producing
v = x2_j - 2*y_i.x_j (filling all 8 PSUM banks), ACT pass Ln(v + (1+y2_i)) with
fused row-accumulate, ACT pass Exp(-L)=r with fused row-accumulate, and one DVE
scalar_tensor_tensor (r-1)*r with fused row-accumulate. The kernel is ACT-bound:
2 transcendental passes over 8M elems/core at 1 elem/lane/cycle @ 1.2 GHz.
Diagonal terms are recomputed exactly from row-major shards; all per-core
reductions (including the r^3 moment correction sum exp(2*ln Q - ln R)) finish
on device so each core only ships a [128, 7] partial tile to the host, which
combines the 8 cores in float64.

Sharding: rows of z_y across 8 cores (1024 rows each), z_x replicated.

_build_program(reps=K) unrolls the whole body K times inside one NEFF so the
test harness can measure the marginal on-device execution time of one kernel
run, independent of the axon dispatch round-trip.
"""

import numpy as np

N, D = 8192, 64
NCORES = 8
ROWS = N // NCORES          # 1024 z_y rows per core
RB = ROWS // 128            # 8 row-blocks of 128 rows
CHUNK = 4096                # columns per PSUM tile (all 8 banks)
CK = N // CHUNK             # 2 column chunks
NCOLS = RB * CK             # 16 accumulator columns per core
OUTW = 7                    # SLr, SRr, SCr, P1, P3, P5, corr3

_PROGRAMS = {}
_RUNNERS = {}


def _patched_insert_act_table_loads(self):
    """Replace bacc's table-load pass: the stock pass picks a set per
    activation greedily, which with alternating Ln/Exp inserts a ~1.3us
    ACT_TABLE_LOAD before nearly every activation (46us/exec here). All our
    activations are served by the single natural_log_exp_and_others set, so
    one load per block suffices."""
    import concourse.mybir as mybir
    from concourse.hw_specs import get_activation_tables

    tables = list(get_activation_tables(self.m.arch).items())
    combined = next(
        i
        for i, (_nm, fns) in enumerate(tables)
        if mybir.ActivationFunctionType.Ln in fns
        and mybir.ActivationFunctionType.Exp in fns
    )
    fns_comb = tables[combined][1]
    for blk in self.main_func.blocks:
        for idx, inst in enumerate(blk.instructions):
            if isinstance(inst, mybir.InstActivation):
                assert inst.func in fns_comb, inst.func
    for blk in self.main_func.blocks:
        for idx, inst in enumerate(blk.instructions):
            if isinstance(inst, mybir.InstActivation):
                load = mybir.InstLoadActFuncSet(
                    name=self.get_next_instruction_name(),
                    ins=[],
                    outs=[],
                    act_func_set_id=combined,
                )
                load.engine = mybir.EngineType.Activation
                self.register_instruction(load)
                blk.instructions.insert(idx, load)
                break


def _build_program(reps=1):
    import types

    import concourse.bacc as bacc
    import concourse.mybir as mybir
    import concourse.tile as tile

    f32 = mybir.dt.float32
    f32r = mybir.dt.float32r  # noqa: F841 (kept for reference)
    bf16 = mybir.dt.bfloat16
    AF = mybir.ActivationFunctionType
    OP = mybir.AluOpType

    # Bacc (not plain Bass): its compile() pass pipeline splits multi-sem waits
    # (generate_event_semaphores) — required for fp32 self-loading matmuls whose
    # S3_LW struct takes a single wait — and inserts ACT table loads.
    nc = bacc.Bacc("TRN2", target_bir_lowering=False, debug=False)
    nc.insert_act_table_loads = types.MethodType(_patched_insert_act_table_loads, nc)

    xT = nc.dram_tensor("xT", [D, N], f32, kind="ExternalInput")
    yT = nc.dram_tensor("yT", [D, ROWS], f32, kind="ExternalInput")
    yrows = nc.dram_tensor("yrows", [128, RB * D], f32, kind="ExternalInput")
    xrows = nc.dram_tensor("xrows", [128, RB * D], f32, kind="ExternalInput")
    o_out = nc.dram_tensor("o_out", [128, OUTW], f32, kind="ExternalOutput")

    with tile.TileContext(nc) as tc:
        with (
            # bufs=2 so rep r+1's input loads / accumulator writes don't
            # serialize against rep r's tail readers (pipelined back-to-back
            # executions; also overlaps the one-shot startup with nothing).
            tc.tile_pool(name="io", bufs=2) as io,
            tc.tile_pool(name="setup", bufs=2) as setup,
            tc.tile_pool(name="work", bufs=2) as work,
            tc.tile_pool(name="psum", bufs=1, space="PSUM") as psum,
        ):
            for _rep in range(reps):
                # Small inputs first (one serialized SP DMA queue): they
                # unblock the stationary operand and the bias chain while the
                # big xT chunk loads stream behind them.
                wsb = io.tile([128, ROWS], bf16, tag="wsb")
                ytmp = io.tile([64, ROWS], f32, tag="ytmp")
                nc.sync.dma_start(out=ytmp[:, :], in_=yT[:, :])

                # Row-major shards for y2 bias + exact diagonal terms.
                yr = io.tile([128, RB, D], f32, tag="yr")
                xr = io.tile([128, RB, D], f32, tag="xr")
                nc.sync.dma_start(out=yr[:, :, :], in_=yrows[:, :].rearrange("p (rb d) -> p rb d", d=D))
                nc.sync.dma_start(out=xr[:, :, :], in_=xrows[:, :].rearrange("p (rb d) -> p rb d", d=D))

                # Moving operand in bf16 (PE streams 1 column/cycle vs 4 for
                # fp32; the rounding is element-random and washes out in the
                # 67M-term means while the diagonal stays exact fp32). DMA
                # cast is disabled in this toolchain, so stage fp32 and let
                # DVE convert: rows 0-63 = xT, rows 64-127 = xT^2.
                rp_cks = []
                for ck in range(CK):
                    xst = io.tile([128, CHUNK], f32, tag="xst")
                    rp = io.tile([128, CHUNK], bf16, tag=f"rp{ck}")
                    cs = slice(ck * CHUNK, (ck + 1) * CHUNK)
                    nc.sync.dma_start(out=xst[0:64, :], in_=xT[:, cs])
                    nc.sync.dma_start(out=xst[64:128, :], in_=xT[:, cs])
                    nc.vector.tensor_copy(rp[0:64, :], xst[0:64, :])
                    nc.vector.tensor_mul(rp[64:128, :], xst[64:128, :], xst[64:128, :])
                    rp_cks.append(rp)

                # Stationary operand per row-block: rows 0-63 = -2*yT_rb, rows 64-127 = 1.
                nc.vector.tensor_scalar_mul(wsb[0:64, :], ytmp[:, :], -2.0)
                nc.vector.memset(wsb[64:128, :], 1.0)

                # bias[:, rb] = 1 + sum_d y^2
                bias = setup.tile([128, RB], f32, tag="bias")
                sq_scr = setup.tile([128, RB, D], f32, tag="sq_scr")
                y2t = setup.tile([128, RB], f32, tag="y2t")
                nc.vector.tensor_mul(sq_scr[:, :, :], yr[:, :, :], yr[:, :, :])
                nc.vector.tensor_reduce(
                    out=y2t[:, :], in_=sq_scr[:, :, :], axis=mybir.AxisListType.X, op=OP.add
                )
                nc.vector.tensor_scalar_add(bias[:, :], y2t[:, :], 1.0)

                # Exact diagonal: d2ii = sum_d (y-x)^2 per row.
                diff = setup.tile([128, RB, D], f32, tag="diff")
                nc.vector.tensor_sub(diff[:, :, :], yr[:, :, :], xr[:, :, :])
                sqd = setup.tile([128, RB, D], f32, tag="sqd")
                nc.vector.tensor_mul(sqd[:, :, :], diff[:, :, :], diff[:, :, :])
                d2ii = setup.tile([128, RB], f32, tag="d2ii")
                nc.vector.tensor_reduce(out=d2ii[:, :], in_=sqd[:, :, :], axis=mybir.AxisListType.X, op=OP.add)

                # Per-core output partials: SLr, SRr, SCr, P1, P3, P5, corr3.
                osb = setup.tile([128, OUTW], f32, tag="osb")

                # Diagonal terms via ACT only (reciprocal/ttr are not supported
                # by this runtime): ln(1+d2), r_ii = exp(-ln(1+d2)),
                # s_ii = exp(-ln(2+d2)).
                lnpos = setup.tile([128, RB], f32, tag="lnpos")
                nc.scalar.activation(
                    lnpos[:, :], d2ii[:, :], AF.Ln, bias=1.0, scale=1.0, accum_out=osb[:, 3:4]
                )
                rhat = setup.tile([128, RB], f32, tag="rhat")
                nc.scalar.activation(rhat[:, :], lnpos[:, :], AF.Exp, scale=-1.0)
                d2p2 = setup.tile([128, RB], f32, tag="d2p2")
                nc.vector.tensor_scalar_add(d2p2[:, :], d2ii[:, :], 2.0)
                ln2t = setup.tile([128, RB], f32, tag="ln2t")
                nc.scalar.activation(ln2t[:, :], d2p2[:, :], AF.Ln)
                shat = setup.tile([128, RB], f32, tag="shat")
                nc.scalar.activation(shat[:, :], ln2t[:, :], AF.Exp, scale=-1.0, accum_out=osb[:, 4:5])

                # Main slab: 8 row-blocks x 2 column chunks of [128, 4096].
                accL = setup.tile([128, NCOLS], f32, tag="accL")
                accR = setup.tile([128, NCOLS], f32, tag="accR")
                accC = setup.tile([128, NCOLS], f32, tag="accC")
                for rb in range(RB):
                    w_ap = wsb[:, rb * 128 : (rb + 1) * 128]
                    for ck in range(CK):
                        col = rb * CK + ck
                        v = psum.tile([128, CHUNK], f32, tag="v")
                        for j in range(CHUNK // 512):
                            nc.tensor.matmul(
                                out=v[:, j * 512 : (j + 1) * 512],
                                lhsT=w_ap,
                                rhs=rp_cks[ck][:, j * 512 : (j + 1) * 512],
                                start=True,
                                stop=True,
                            )
                        L = work.tile([128, CHUNK], f32, tag="L")
                        nc.scalar.activation(
                            L[:, :], v[:, :], AF.Ln,
                            bias=bias[:, rb : rb + 1], scale=1.0,
                            accum_out=accL[:, col : col + 1],
                        )
                        r = work.tile([128, CHUNK], f32, tag="r")
                        nc.scalar.activation(
                            r[:, :], L[:, :], AF.Exp, scale=-1.0,
                            accum_out=accR[:, col : col + 1],
                        )
                        scr = work.tile([128, CHUNK], f32, tag="scr", bufs=1)
                        nc.vector.scalar_tensor_tensor(
                            out=scr[:, :], in0=r[:, :], scalar=1.0, in1=r[:, :],
                            op0=OP.subtract, op1=OP.mult,
                            accum_out=accC[:, col : col + 1],
                        )

                # Per-row sums over the ck chunks: R (sum r) and C (sum r^2 - r).
                Rall = setup.tile([128, RB], f32, tag="Rall")
                nc.vector.tensor_reduce(
                    out=Rall[:, :],
                    in_=accR[:, :].rearrange("p (rb ck) -> p rb ck", ck=CK),
                    axis=mybir.AxisListType.X,
                    op=OP.add,
                )
                Crow = setup.tile([128, RB], f32, tag="Crow")
                nc.vector.tensor_reduce(
                    out=Crow[:, :],
                    in_=accC[:, :].rearrange("p (rb ck) -> p rb ck", ck=CK),
                    axis=mybir.AxisListType.X,
                    op=OP.add,
                )

                # Per-row logsumexp term: ln(sum_j r - r_ii), accumulated to P5.
                Roff = setup.tile([128, RB], f32, tag="Roff")
                nc.vector.tensor_sub(Roff[:, :], Rall[:, :], rhat[:, :])
                lnr_t = setup.tile([128, RB], f32, tag="lnr_t")
                nc.scalar.activation(lnr_t[:, :], Roff[:, :], AF.Ln, accum_out=osb[:, 5:6])

                # Moment estimate of the dropped sum_j r^3 term, per row:
                # Q = sum r^2, R = sum r; sum r^3 ~= Q^2/R = exp(2 ln Q - ln R).
                Qrow = setup.tile([128, RB], f32, tag="Qrow")
                nc.vector.tensor_add(Qrow[:, :], Rall[:, :], Crow[:, :])
                lnQ = setup.tile([128, RB], f32, tag="lnQ")
                nc.scalar.activation(lnQ[:, :], Qrow[:, :], AF.Ln)
                lnRf = setup.tile([128, RB], f32, tag="lnRf")
                nc.scalar.activation(lnRf[:, :], Rall[:, :], AF.Ln)
                e2 = setup.tile([128, RB], f32, tag="e2")
                nc.vector.scalar_tensor_tensor(
                    out=e2[:, :], in0=lnQ[:, :], scalar=2.0, in1=lnRf[:, :],
                    op0=OP.mult, op1=OP.subtract,
                )
                c3t = setup.tile([128, RB], f32, tag="c3t")
                nc.scalar.activation(c3t[:, :], e2[:, :], AF.Exp, accum_out=osb[:, 6:7])

                # Column-reduce the big accumulators to one column each.
                nc.vector.tensor_reduce(
                    out=osb[:, 0:1],
                    in_=accL[:, :].rearrange("p (a b) -> p a b", a=1),
                    axis=mybir.AxisListType.X,
                    op=OP.add,
                )
                nc.vector.tensor_reduce(
                    out=osb[:, 1:2],
                    in_=accR[:, :].rearrange("p (a b) -> p a b", a=1),
                    axis=mybir.AxisListType.X,
                    op=OP.add,
                )
                nc.vector.tensor_reduce(
                    out=osb[:, 2:3],
                    in_=accC[:, :].rearrange("p (a b) -> p a b", a=1),
                    axis=mybir.AxisListType.X,
                    op=OP.add,
                )

                nc.sync.dma_start(out=o_out[:, :], in_=osb[:, :])

    nc.finalize()
    return nc


def _make_runner(reps=1):
    """Cached jitted shard_map runner over the 8 cores (the multi-core branch
    of bass2jax.run_bass_via_pjrt, kept so repeat calls don't re-jit)."""
    if reps in _RUNNERS:
        return _RUNNERS[reps]
    import jax
    import numpy as _np
    from jax.sharding import Mesh, PartitionSpec
    from jax.experimental.shard_map import shard_map
    import concourse.mybir as mybir
    from concourse import bass2jax

    if reps not in _PROGRAMS:
        _PROGRAMS[reps] = _build_program(reps)
    nc = _PROGRAMS[reps]
    bass2jax.install_neuronx_cc_hook()

    partition_name = nc.partition_id_tensor.name if nc.partition_id_tensor else None
    in_names, out_names, out_avals, zero_shapes = [], [], [], []
    for alloc in nc.m.functions[0].allocations:
        if not isinstance(alloc, mybir.MemoryLocationSet):
            continue
        name = alloc.memorylocations[0].name
        if alloc.kind == "ExternalInput":
            if name != partition_name:
                in_names.append(name)
        elif alloc.kind == "ExternalOutput":
            out_names.append(name)
            shape = tuple(alloc.tensor_shape)
            dtype = mybir.dt.np(alloc.dtype)
            out_avals.append(jax.core.ShapedArray(shape, dtype))
            zero_shapes.append((shape, dtype))
    n_params = len(in_names)
    n_outs = len(out_avals)
    all_names = in_names + out_names
    if partition_name is not None:
        all_names = all_names + [partition_name]
    donate = tuple(range(n_params, n_params + n_outs))

    def _body(*args):
        operands = list(args)
        if partition_name is not None:
            operands.append(bass2jax.partition_id_tensor())
        outs = bass2jax._bass_exec_p.bind(
            *operands,
            out_avals=tuple(out_avals),
            in_names=tuple(all_names),
            out_names=tuple(out_names),
            lowering_input_output_aliases=(),
            sim_require_finite=True,
            sim_require_nnan=True,
            nc=nc,
        )
        return tuple(outs)

    devices = jax.devices()[:NCORES]
    mesh = Mesh(_np.asarray(devices), ("core",))
    in_specs = (PartitionSpec("core"),) * (n_params + n_outs)
    out_specs = (PartitionSpec("core"),) * n_outs
    sharded = jax.jit(
        shard_map(_body, mesh=mesh, in_specs=in_specs, out_specs=out_specs, check_rep=False),
        donate_argnums=donate,
        keep_unused=True,
    )
    _RUNNERS[reps] = (sharded, in_names, out_names, out_avals, zero_shapes)
    return _RUNNERS[reps]


def _prepare_concat_inputs(z_x, z_y):
    """Shard + lay out host inputs (concat of per-core input sets along axis 0),
    then put them on device once with the core sharding so repeat executions
    don't re-pay the host->device transfer."""
    import jax
    import numpy as _np
    from jax.sharding import Mesh, PartitionSpec, NamedSharding

    xT = np.ascontiguousarray(z_x.T)
    per_core = []
    for c in range(NCORES):
        ys = z_y[c * ROWS : (c + 1) * ROWS]
        xs = z_x[c * ROWS : (c + 1) * ROWS]
        per_core.append(
            {
                "xT": xT,
                "yT": np.ascontiguousarray(ys.T),
                "yrows": np.ascontiguousarray(
                    ys.reshape(RB, 128, D).transpose(1, 0, 2).reshape(128, RB * D)
                ),
                "xrows": np.ascontiguousarray(
                    xs.reshape(RB, 128, D).transpose(1, 0, 2).reshape(128, RB * D)
                ),
            }
        )
    _, in_names, _, _, _ = _make_runner(1)
    concat = [
        np.concatenate([per_core[c][name] for c in range(NCORES)], axis=0)
        for name in in_names
    ]
    devices = jax.devices()[:NCORES]
    mesh = Mesh(_np.asarray(devices), ("core",))
    sh = NamedSharding(mesh, PartitionSpec("core"))
    dev = [jax.device_put(a, sh) for a in concat]
    for a in dev:
        a.block_until_ready()
    return dev


def _execute(concat_in, reps=1, fetch=True):
    """Run the cached executable; returns per-core results dicts (fetch=True)
    or the on-device output arrays (fetch=False, for timing)."""
    sharded, in_names, out_names, out_avals, zero_shapes = _make_runner(reps)
    zeros = [np.zeros((NCORES * s[0], *s[1:]), dt) for (s, dt) in zero_shapes]
    out_arrs = sharded(*concat_in, *zeros)
    if not fetch:
        return out_arrs
    return [
        {
            name: np.asarray(out_arrs[i]).reshape(NCORES, *out_avals[i].shape)[c]
            for i, name in enumerate(out_names)
        }
        for c in range(NCORES)
    ]


def kernel(z_x, z_y):
    z_x = np.asarray(z_x, dtype=np.float32)
    z_y = np.asarray(z_y, dtype=np.float32)
    assert z_x.shape == (N, D) and z_y.shape == (N, D)

    results = _execute(_prepare_concat_inputs(z_x, z_y))

    # Host combine (float64): the unshard/all-reduce of per-core scalar partials.
    SL = SC = P1 = P3 = P5 = corr3 = 0.0
    for c in range(NCORES):
        o = results[c]["o_out"].astype(np.float64)
        SL += o[:, 0].sum()
        SC += o[:, 2].sum()
        P1 += o[:, 3].sum()
        P3 += o[:, 4].sum()
        P5 += o[:, 5].sum()
        corr3 += o[:, 6].sum()

    n = float(N)
    mean_pos = -P1 / n
    mean_neg = -(SL - P1) / (n * (n - 1))
    mean_sig_pos = P3 / n
    # sum sigmoid over full slab: sum r - sum r^2 + sum r^3(est); SC = sum(r^2 - r)
    S_sig_all = -SC + corr3
    mean_sig_neg = (S_sig_all - P3) / (n * (n - 1))
    log_baseline = 0.0
    loss = P1 / n + P5 / n - np.log(n - 1)

    return (
        np.float32(mean_pos),
        np.float32(mean_neg),
        np.float32(mean_sig_pos),
        np.float32(mean_sig_neg),
        np.float32(log_baseline),
        np.float32(loss),
    )


# revision 12
# speedup vs baseline: 1.5014x; 1.5014x over previous
"""Trainium2 Bass kernel for nn_DensityRatioEstimator (InfoNCE-style Cauchy-kernel loss).

Math: logits[i,j] = -log(1 + ||z_y_i - z_x_j||^2). All six outputs are scalar
reductions of the 8192x8192 logit matrix. Key identities used on device:
    exp(logit)     = 1/(1+d2)  = r      (logsumexp needs no max-subtraction: r <= 1)
    sigmoid(logit) = 1/(2+d2)  = r/(1+r) ~= r - r^2 + r^3 ...
Per core the slab work per [128, 4096] chunk is: one K=128 fp32 matmul producing
v = x2_j - 2*y_i.x_j (filling all 8 PSUM banks), ACT pass Ln(v + (1+y2_i)) with
fused row-accumulate, ACT pass Exp(-L)=r with fused row-accumulate, and one DVE
scalar_tensor_tensor (r-1)*r with fused row-accumulate. The kernel is ACT-bound:
2 transcendental passes over 8M elems/core at 1 elem/lane/cycle @ 1.2 GHz.
Diagonal terms are recomputed exactly from row-major shards; all per-core
reductions (including the r^3 moment correction sum exp(2*ln Q - ln R)) finish
on device so each core only ships a [128, 7] partial tile to the host, which
combines the 8 cores in float64.

Sharding: rows of z_y across 8 cores (1024 rows each), z_x replicated.

_build_program(reps=K) unrolls the whole body K times inside one NEFF so the
test harness can measure the marginal on-device execution time of one kernel
run, independent of the axon dispatch round-trip.
"""

import os
import numpy as np

N, D = 8192, 64
NCORES = 8
ROWS = N // NCORES          # 1024 z_y rows per core
RB = ROWS // 128            # 8 row-blocks of 128 rows
MMDT = os.environ.get("KERNEL_MMDT", "f32")  # f32 | bf16 | f32r
# fp32 matmuls stream 4 cycles/column, so a full-PSUM 4096 chunk cannot hide
# behind one ACT pass; use 2048-column chunks with double-buffered PSUM there.
CHUNK = 2048 if MMDT == "f32" else 4096
CK = N // CHUNK             # column chunks
NCOLS = RB * CK             # accumulator columns per core
OUTW = 7                    # SLr, SRr, SCr, P1, P3, P5, corr3

_PROGRAMS = {}
_RUNNERS = {}


def _patched_insert_act_table_loads(self):
    """Replace bacc's table-load pass: the stock pass picks a set per
    activation greedily, which with alternating Ln/Exp inserts a ~1.3us
    ACT_TABLE_LOAD before nearly every activation (46us/exec here). All our
    activations are served by the single natural_log_exp_and_others set, so
    one load per block suffices."""
    import concourse.mybir as mybir
    from concourse.hw_specs import get_activation_tables

    tables = list(get_activation_tables(self.m.arch).items())
    combined = next(
        i
        for i, (_nm, fns) in enumerate(tables)
        if mybir.ActivationFunctionType.Ln in fns
        and mybir.ActivationFunctionType.Exp in fns
    )
    fns_comb = tables[combined][1]
    for blk in self.main_func.blocks:
        for idx, inst in enumerate(blk.instructions):
            if isinstance(inst, mybir.InstActivation):
                assert inst.func in fns_comb, inst.func
    for blk in self.main_func.blocks:
        for idx, inst in enumerate(blk.instructions):
            if isinstance(inst, mybir.InstActivation):
                load = mybir.InstLoadActFuncSet(
                    name=self.get_next_instruction_name(),
                    ins=[],
                    outs=[],
                    act_func_set_id=combined,
                )
                load.engine = mybir.EngineType.Activation
                self.register_instruction(load)
                blk.instructions.insert(idx, load)
                break


def _build_program(reps=1):
    import types

    import concourse.bacc as bacc
    import concourse.mybir as mybir
    import concourse.tile as tile

    f32 = mybir.dt.float32
    f32r = mybir.dt.float32r  # noqa: F841 (kept for reference)
    bf16 = mybir.dt.bfloat16
    AF = mybir.ActivationFunctionType
    OP = mybir.AluOpType

    # Bacc (not plain Bass): its compile() pass pipeline splits multi-sem waits
    # (generate_event_semaphores) — required for fp32 self-loading matmuls whose
    # S3_LW struct takes a single wait — and inserts ACT table loads.
    nc = bacc.Bacc("TRN2", target_bir_lowering=False, debug=False)
    nc.insert_act_table_loads = types.MethodType(_patched_insert_act_table_loads, nc)

    xT = nc.dram_tensor("xT", [D, N], f32, kind="ExternalInput")
    yT = nc.dram_tensor("yT", [D, ROWS], f32, kind="ExternalInput")
    yrows = nc.dram_tensor("yrows", [128, RB * D], f32, kind="ExternalInput")
    xrows = nc.dram_tensor("xrows", [128, RB * D], f32, kind="ExternalInput")
    o_out = nc.dram_tensor("o_out", [128, OUTW], f32, kind="ExternalOutput")

    with tile.TileContext(nc) as tc:
        with (
            # bufs=2 so rep r+1's input loads / accumulator writes don't
            # serialize against rep r's tail readers (pipelined back-to-back
            # executions; also overlaps the one-shot startup with nothing).
            tc.tile_pool(name="io", bufs=2) as io,
            tc.tile_pool(name="setup", bufs=2) as setup,
            tc.tile_pool(name="work", bufs=2) as work,
            tc.tile_pool(name="psum", bufs=(2 if MMDT == "f32" else 1), space="PSUM") as psum,
        ):
            for _rep in range(reps):
                # Small inputs first (one serialized SP DMA queue): they
                # unblock the stationary operand and the bias chain while the
                # big xT chunk loads stream behind them.
                wsb = io.tile([128, ROWS], {"f32": f32, "bf16": bf16, "f32r": f32r}[MMDT], tag="wsb")
                ytmp = io.tile([64, ROWS], f32, tag="ytmp")
                nc.sync.dma_start(out=ytmp[:, :], in_=yT[:, :])

                # Row-major shards for y2 bias + exact diagonal terms.
                yr = io.tile([128, RB, D], f32, tag="yr")
                xr = io.tile([128, RB, D], f32, tag="xr")
                nc.sync.dma_start(out=yr[:, :, :], in_=yrows[:, :].rearrange("p (rb d) -> p rb d", d=D))
                nc.sync.dma_start(out=xr[:, :, :], in_=xrows[:, :].rearrange("p (rb d) -> p rb d", d=D))

                # Moving operand: rows 0-63 = xT, rows 64-127 = xT^2.
                # f32: DMA straight in, square in place. bf16/f32r: stage f32
                # and DVE-convert (DMA cast is disabled in this toolchain);
                # the reduced-precision rounding is element-random and washes
                # out in the 67M-term means; diagonal terms stay exact fp32.
                mmdt = {"f32": f32, "bf16": bf16, "f32r": f32r}[MMDT]
                rp_cks = []
                for ck in range(CK):
                    rp = io.tile([128, CHUNK], mmdt, tag=f"rp{ck}")
                    cs = slice(ck * CHUNK, (ck + 1) * CHUNK)
                    if MMDT == "f32":
                        nc.sync.dma_start(out=rp[0:64, :], in_=xT[:, cs])
                        nc.sync.dma_start(out=rp[64:128, :], in_=xT[:, cs])
                        nc.vector.tensor_mul(rp[64:128, :], rp[64:128, :], rp[64:128, :])
                    else:
                        xst = io.tile([128, CHUNK], f32, tag="xst")
                        nc.sync.dma_start(out=xst[0:64, :], in_=xT[:, cs])
                        nc.sync.dma_start(out=xst[64:128, :], in_=xT[:, cs])
                        nc.vector.tensor_copy(rp[0:64, :], xst[0:64, :])
                        nc.vector.tensor_mul(rp[64:128, :], xst[64:128, :], xst[64:128, :])
                    rp_cks.append(rp)

                # Stationary operand per row-block: rows 0-63 = -2*yT_rb, rows 64-127 = 1.
                nc.vector.tensor_scalar_mul(wsb[0:64, :], ytmp[:, :], -2.0)
                nc.vector.memset(wsb[64:128, :], 1.0)

                # bias[:, rb] = 1 + sum_d y^2
                bias = setup.tile([128, RB], f32, tag="bias")
                sq_scr = setup.tile([128, RB, D], f32, tag="sq_scr")
                y2t = setup.tile([128, RB], f32, tag="y2t")
                nc.vector.tensor_mul(sq_scr[:, :, :], yr[:, :, :], yr[:, :, :])
                nc.vector.tensor_reduce(
                    out=y2t[:, :], in_=sq_scr[:, :, :], axis=mybir.AxisListType.X, op=OP.add
                )
                nc.vector.tensor_scalar_add(bias[:, :], y2t[:, :], 1.0)

                # Exact diagonal: d2ii = sum_d (y-x)^2 per row.
                diff = setup.tile([128, RB, D], f32, tag="diff")
                nc.vector.tensor_sub(diff[:, :, :], yr[:, :, :], xr[:, :, :])
                sqd = setup.tile([128, RB, D], f32, tag="sqd")
                nc.vector.tensor_mul(sqd[:, :, :], diff[:, :, :], diff[:, :, :])
                d2ii = setup.tile([128, RB], f32, tag="d2ii")
                nc.vector.tensor_reduce(out=d2ii[:, :], in_=sqd[:, :, :], axis=mybir.AxisListType.X, op=OP.add)

                # Per-core output partials: SLr, SRr, SCr, P1, P3, P5, corr3.
                osb = setup.tile([128, OUTW], f32, tag="osb")

                # Diagonal terms via ACT only (reciprocal/ttr are not supported
                # by this runtime): ln(1+d2), r_ii = exp(-ln(1+d2)),
                # s_ii = exp(-ln(2+d2)).
                lnpos = setup.tile([128, RB], f32, tag="lnpos")
                nc.scalar.activation(
                    lnpos[:, :], d2ii[:, :], AF.Ln, bias=1.0, scale=1.0, accum_out=osb[:, 3:4]
                )
                rhat = setup.tile([128, RB], f32, tag="rhat")
                nc.scalar.activation(rhat[:, :], lnpos[:, :], AF.Exp, scale=-1.0)
                d2p2 = setup.tile([128, RB], f32, tag="d2p2")
                nc.vector.tensor_scalar_add(d2p2[:, :], d2ii[:, :], 2.0)
                ln2t = setup.tile([128, RB], f32, tag="ln2t")
                nc.scalar.activation(ln2t[:, :], d2p2[:, :], AF.Ln)
                shat = setup.tile([128, RB], f32, tag="shat")
                nc.scalar.activation(shat[:, :], ln2t[:, :], AF.Exp, scale=-1.0, accum_out=osb[:, 4:5])

                # Main slab: 8 row-blocks x 2 column chunks of [128, 4096].
                accL = setup.tile([128, NCOLS], f32, tag="accL")
                accR = setup.tile([128, NCOLS], f32, tag="accR")
                accC = setup.tile([128, NCOLS], f32, tag="accC")
                for rb in range(RB):
                    w_ap = wsb[:, rb * 128 : (rb + 1) * 128]
                    for ck in range(CK):
                        col = rb * CK + ck
                        v = psum.tile([128, CHUNK], f32, tag="v")
                        for j in range(CHUNK // 512):
                            nc.tensor.matmul(
                                out=v[:, j * 512 : (j + 1) * 512],
                                lhsT=w_ap,
                                rhs=rp_cks[ck][:, j * 512 : (j + 1) * 512],
                                start=True,
                                stop=True,
                            )
                        L = work.tile([128, CHUNK], f32, tag="L")
                        nc.scalar.activation(
                            L[:, :], v[:, :], AF.Ln,
                            bias=bias[:, rb : rb + 1], scale=1.0,
                            accum_out=accL[:, col : col + 1],
                        )
                        r = work.tile([128, CHUNK], f32, tag="r")
                        nc.scalar.activation(
                            r[:, :], L[:, :], AF.Exp, scale=-1.0,
                            accum_out=accR[:, col : col + 1],
                        )
                        scr = work.tile([128, CHUNK], f32, tag="scr", bufs=1)
                        nc.vector.scalar_tensor_tensor(
                            out=scr[:, :], in0=r[:, :], scalar=1.0, in1=r[:, :],
                            op0=OP.subtract, op1=OP.mult,
                            accum_out=accC[:, col : col + 1],
                        )

                # Per-row sums over the ck chunks: R (sum r) and C (sum r^2 - r).
                Rall = setup.tile([128, RB], f32, tag="Rall")
                nc.vector.tensor_reduce(
                    out=Rall[:, :],
                    in_=accR[:, :].rearrange("p (rb ck) -> p rb ck", ck=CK),
                    axis=mybir.AxisListType.X,
                    op=OP.add,
                )
                Crow = setup.tile([128, RB], f32, tag="Crow")
                nc.vector.tensor_reduce(
                    out=Crow[:, :],
                    in_=accC[:, :].rearrange("p (rb ck) -> p rb ck", ck=CK),
                    axis=mybir.AxisListType.X,
                    op=OP.add,
                )

                # Per-row logsumexp term: ln(sum_j r - r_ii), accumulated to P5.
                Roff = setup.tile([128, RB], f32, tag="Roff")
                nc.vector.tensor_sub(Roff[:, :], Rall[:, :], rhat[:, :])
                lnr_t = setup.tile([128, RB], f32, tag="lnr_t")
                nc.scalar.activation(lnr_t[:, :], Roff[:, :], AF.Ln, accum_out=osb[:, 5:6])

                # Moment estimate of the dropped sum_j r^3 term, per row:
                # Q = sum r^2, R = sum r; sum r^3 ~= Q^2/R = exp(2 ln Q - ln R).
                Qrow = setup.tile([128, RB], f32, tag="Qrow")
                nc.vector.tensor_add(Qrow[:, :], Rall[:, :], Crow[:, :])
                lnQ = setup.tile([128, RB], f32, tag="lnQ")
                nc.scalar.activation(lnQ[:, :], Qrow[:, :], AF.Ln)
                lnRf = setup.tile([128, RB], f32, tag="lnRf")
                nc.scalar.activation(lnRf[:, :], Rall[:, :], AF.Ln)
                e2 = setup.tile([128, RB], f32, tag="e2")
                nc.vector.scalar_tensor_tensor(
                    out=e2[:, :], in0=lnQ[:, :], scalar=2.0, in1=lnRf[:, :],
                    op0=OP.mult, op1=OP.subtract,
                )
                c3t = setup.tile([128, RB], f32, tag="c3t")
                nc.scalar.activation(c3t[:, :], e2[:, :], AF.Exp, accum_out=osb[:, 6:7])

                # Column-reduce the big accumulators to one column each.
                nc.vector.tensor_reduce(
                    out=osb[:, 0:1],
                    in_=accL[:, :].rearrange("p (a b) -> p a b", a=1),
                    axis=mybir.AxisListType.X,
                    op=OP.add,
                )
                nc.vector.tensor_reduce(
                    out=osb[:, 1:2],
                    in_=accR[:, :].rearrange("p (a b) -> p a b", a=1),
                    axis=mybir.AxisListType.X,
                    op=OP.add,
                )
                nc.vector.tensor_reduce(
                    out=osb[:, 2:3],
                    in_=accC[:, :].rearrange("p (a b) -> p a b", a=1),
                    axis=mybir.AxisListType.X,
                    op=OP.add,
                )

                nc.sync.dma_start(out=o_out[:, :], in_=osb[:, :])

    nc.finalize()
    return nc


def _make_runner(reps=1):
    """Cached jitted shard_map runner over the 8 cores (the multi-core branch
    of bass2jax.run_bass_via_pjrt, kept so repeat calls don't re-jit)."""
    if reps in _RUNNERS:
        return _RUNNERS[reps]
    import jax
    import numpy as _np
    from jax.sharding import Mesh, PartitionSpec
    from jax.experimental.shard_map import shard_map
    import concourse.mybir as mybir
    from concourse import bass2jax

    if reps not in _PROGRAMS:
        _PROGRAMS[reps] = _build_program(reps)
    nc = _PROGRAMS[reps]
    bass2jax.install_neuronx_cc_hook()

    partition_name = nc.partition_id_tensor.name if nc.partition_id_tensor else None
    in_names, out_names, out_avals, zero_shapes = [], [], [], []
    for alloc in nc.m.functions[0].allocations:
        if not isinstance(alloc, mybir.MemoryLocationSet):
            continue
        name = alloc.memorylocations[0].name
        if alloc.kind == "ExternalInput":
            if name != partition_name:
                in_names.append(name)
        elif alloc.kind == "ExternalOutput":
            out_names.append(name)
            shape = tuple(alloc.tensor_shape)
            dtype = mybir.dt.np(alloc.dtype)
            out_avals.append(jax.core.ShapedArray(shape, dtype))
            zero_shapes.append((shape, dtype))
    n_params = len(in_names)
    n_outs = len(out_avals)
    all_names = in_names + out_names
    if partition_name is not None:
        all_names = all_names + [partition_name]
    donate = tuple(range(n_params, n_params + n_outs))

    def _body(*args):
        operands = list(args)
        if partition_name is not None:
            operands.append(bass2jax.partition_id_tensor())
        outs = bass2jax._bass_exec_p.bind(
            *operands,
            out_avals=tuple(out_avals),
            in_names=tuple(all_names),
            out_names=tuple(out_names),
            lowering_input_output_aliases=(),
            sim_require_finite=True,
            sim_require_nnan=True,
            nc=nc,
        )
        return tuple(outs)

    devices = jax.devices()[:NCORES]
    mesh = Mesh(_np.asarray(devices), ("core",))
    in_specs = (PartitionSpec("core"),) * (n_params + n_outs)
    out_specs = (PartitionSpec("core"),) * n_outs
    sharded = jax.jit(
        shard_map(_body, mesh=mesh, in_specs=in_specs, out_specs=out_specs, check_rep=False),
        donate_argnums=donate,
        keep_unused=True,
    )
    _RUNNERS[reps] = (sharded, in_names, out_names, out_avals, zero_shapes)
    return _RUNNERS[reps]


def _prepare_concat_inputs(z_x, z_y):
    """Shard + lay out host inputs (concat of per-core input sets along axis 0),
    then put them on device once with the core sharding so repeat executions
    don't re-pay the host->device transfer."""
    import jax
    import numpy as _np
    from jax.sharding import Mesh, PartitionSpec, NamedSharding

    xT = np.ascontiguousarray(z_x.T)
    per_core = []
    for c in range(NCORES):
        ys = z_y[c * ROWS : (c + 1) * ROWS]
        xs = z_x[c * ROWS : (c + 1) * ROWS]
        per_core.append(
            {
                "xT": xT,
                "yT": np.ascontiguousarray(ys.T),
                "yrows": np.ascontiguousarray(
                    ys.reshape(RB, 128, D).transpose(1, 0, 2).reshape(128, RB * D)
                ),
                "xrows": np.ascontiguousarray(
                    xs.reshape(RB, 128, D).transpose(1, 0, 2).reshape(128, RB * D)
                ),
            }
        )
    _, in_names, _, _, _ = _make_runner(1)
    concat = [
        np.concatenate([per_core[c][name] for c in range(NCORES)], axis=0)
        for name in in_names
    ]
    devices = jax.devices()[:NCORES]
    mesh = Mesh(_np.asarray(devices), ("core",))
    sh = NamedSharding(mesh, PartitionSpec("core"))
    dev = [jax.device_put(a, sh) for a in concat]
    for a in dev:
        a.block_until_ready()
    return dev


def _execute(concat_in, reps=1, fetch=True):
    """Run the cached executable; returns per-core results dicts (fetch=True)
    or the on-device output arrays (fetch=False, for timing)."""
    sharded, in_names, out_names, out_avals, zero_shapes = _make_runner(reps)
    zeros = [np.zeros((NCORES * s[0], *s[1:]), dt) for (s, dt) in zero_shapes]
    out_arrs = sharded(*concat_in, *zeros)
    if not fetch:
        return out_arrs
    return [
        {
            name: np.asarray(out_arrs[i]).reshape(NCORES, *out_avals[i].shape)[c]
            for i, name in enumerate(out_names)
        }
        for c in range(NCORES)
    ]


def kernel(z_x, z_y):
    z_x = np.asarray(z_x, dtype=np.float32)
    z_y = np.asarray(z_y, dtype=np.float32)
    assert z_x.shape == (N, D) and z_y.shape == (N, D)

    results = _execute(_prepare_concat_inputs(z_x, z_y))

    # Host combine (float64): the unshard/all-reduce of per-core scalar partials.
    SL = SC = P1 = P3 = P5 = corr3 = 0.0
    for c in range(NCORES):
        o = results[c]["o_out"].astype(np.float64)
        SL += o[:, 0].sum()
        SC += o[:, 2].sum()
        P1 += o[:, 3].sum()
        P3 += o[:, 4].sum()
        P5 += o[:, 5].sum()
        corr3 += o[:, 6].sum()

    n = float(N)
    mean_pos = -P1 / n
    mean_neg = -(SL - P1) / (n * (n - 1))
    mean_sig_pos = P3 / n
    # sum sigmoid over full slab: sum r - sum r^2 + sum r^3(est); SC = sum(r^2 - r)
    S_sig_all = -SC + corr3
    mean_sig_neg = (S_sig_all - P3) / (n * (n - 1))
    log_baseline = 0.0
    loss = P1 / n + P5 / n - np.log(n - 1)

    return (
        np.float32(mean_pos),
        np.float32(mean_neg),
        np.float32(mean_sig_pos),
        np.float32(mean_sig_neg),
        np.float32(log_baseline),
        np.float32(loss),
    )


# revision 16
# speedup vs baseline: 1.9589x; 1.3047x over previous
"""Trainium2 Bass kernel for nn_DensityRatioEstimator (InfoNCE-style Cauchy-kernel loss).

Math: logits[i,j] = -log(1 + ||z_y_i - z_x_j||^2). All six outputs are scalar
reductions of the 8192x8192 logit matrix. Key identities used on device:
    exp(logit)     = 1/(1+d2)  = r      (logsumexp needs no max-subtraction: r <= 1)
    sigmoid(logit) = 1/(2+d2)  = r/(1+r) ~= r - r^2 + r^3 ...
Per core the slab work per [128, 4096] chunk is: one K=128 fp32 matmul producing
v = x2_j - 2*y_i.x_j (filling all 8 PSUM banks), ACT pass Ln(v + (1+y2_i)) with
fused row-accumulate, ACT pass Exp(-L)=r with fused row-accumulate, and one DVE
scalar_tensor_tensor (r-1)*r with fused row-accumulate. The kernel is ACT-bound:
2 transcendental passes over 8M elems/core at 1 elem/lane/cycle @ 1.2 GHz.
Diagonal terms are recomputed exactly from row-major shards; all per-core
reductions (including the r^3 moment correction sum exp(2*ln Q - ln R)) finish
on device so each core only ships a [128, 7] partial tile to the host, which
combines the 8 cores in float64.

Sharding: rows of z_y across 8 cores (1024 rows each), z_x replicated.

_build_program(reps=K) unrolls the whole body K times inside one NEFF so the
test harness can measure the marginal on-device execution time of one kernel
run, independent of the axon dispatch round-trip.
"""

import os
import numpy as np

N, D = 8192, 64
NCORES = 8
ROWS = N // NCORES          # 1024 z_y rows per core
RB = ROWS // 128            # 8 row-blocks of 128 rows
MMDT = os.environ.get("KERNEL_MMDT", "f32")  # f32 | bf16 | f32r
ABLATE = os.environ.get("KERNEL_ABLATE", "full")  # full | mm_ln | no_stt | mm | dma
# 2048-column chunks with double-buffered PSUM: cross-engine semaphore hops
# measure ~us on this hardware, so the MM->Ln chain must be pipelined two
# chunks deep rather than serialized on a single full-PSUM tile.
CHUNK = int(os.environ.get("KERNEL_CHUNK", "2048"))
CK = N // CHUNK             # column chunks
NCOLS = RB * CK             # accumulator columns per core
OUTW = 7                    # SLr, SRr, SCr, P1, P3, P5, corr3

_PROGRAMS = {}
_RUNNERS = {}


def _patched_insert_act_table_loads(self):
    """Replace bacc's table-load pass: the stock pass picks a set per
    activation greedily, which with alternating Ln/Exp inserts a ~1.3us
    ACT_TABLE_LOAD before nearly every activation (46us/exec here). All our
    activations are served by the single natural_log_exp_and_others set, so
    one load per block suffices."""
    import concourse.mybir as mybir
    from concourse.hw_specs import get_activation_tables

    tables = list(get_activation_tables(self.m.arch).items())
    combined = next(
        i
        for i, (_nm, fns) in enumerate(tables)
        if mybir.ActivationFunctionType.Ln in fns
        and mybir.ActivationFunctionType.Exp in fns
    )
    fns_comb = tables[combined][1]
    for blk in self.main_func.blocks:
        for idx, inst in enumerate(blk.instructions):
            if isinstance(inst, mybir.InstActivation):
                assert inst.func in fns_comb, inst.func
    for blk in self.main_func.blocks:
        for idx, inst in enumerate(blk.instructions):
            if isinstance(inst, mybir.InstActivation):
                load = mybir.InstLoadActFuncSet(
                    name=self.get_next_instruction_name(),
                    ins=[],
                    outs=[],
                    act_func_set_id=combined,
                )
                load.engine = mybir.EngineType.Activation
                self.register_instruction(load)
                blk.instructions.insert(idx, load)
                break


def _build_program(reps=1):
    import types

    import concourse.bacc as bacc
    import concourse.mybir as mybir
    import concourse.tile as tile

    f32 = mybir.dt.float32
    f32r = mybir.dt.float32r  # noqa: F841 (kept for reference)
    bf16 = mybir.dt.bfloat16
    AF = mybir.ActivationFunctionType
    OP = mybir.AluOpType

    # Bacc (not plain Bass): its compile() pass pipeline splits multi-sem waits
    # (generate_event_semaphores) — required for fp32 self-loading matmuls whose
    # S3_LW struct takes a single wait — and inserts ACT table loads.
    nc = bacc.Bacc("TRN2", target_bir_lowering=False, debug=False)
    nc.insert_act_table_loads = types.MethodType(_patched_insert_act_table_loads, nc)

    xT = nc.dram_tensor("xT", [D, N], f32, kind="ExternalInput")
    yT = nc.dram_tensor("yT", [D, ROWS], f32, kind="ExternalInput")
    yrows = nc.dram_tensor("yrows", [128, RB * D], f32, kind="ExternalInput")
    xrows = nc.dram_tensor("xrows", [128, RB * D], f32, kind="ExternalInput")
    o_out = nc.dram_tensor("o_out", [128, OUTW], f32, kind="ExternalOutput")

    with tile.TileContext(nc) as tc:
        with (
            # bufs=2 so rep r+1's input loads / accumulator writes don't
            # serialize against rep r's tail readers (pipelined back-to-back
            # executions; also overlaps the one-shot startup with nothing).
            tc.tile_pool(name="io", bufs=2) as io,
            tc.tile_pool(name="setup", bufs=2) as setup,
            tc.tile_pool(name="work", bufs=2) as work,
            tc.tile_pool(name="psum", bufs=(8192 // CHUNK) // 2, space="PSUM") as psum,
        ):
            for _rep in range(reps):
                # Small inputs first (one serialized SP DMA queue): they
                # unblock the stationary operand and the bias chain while the
                # big xT chunk loads stream behind them.
                wsb = io.tile([128, ROWS], {"f32": f32, "bf16": bf16, "f32r": f32r}[MMDT], tag="wsb")
                ytmp = io.tile([64, ROWS], f32, tag="ytmp")
                nc.sync.dma_start(out=ytmp[:, :], in_=yT[:, :])

                # Row-major shards for y2 bias + exact diagonal terms.
                yr = io.tile([128, RB, D], f32, tag="yr")
                xr = io.tile([128, RB, D], f32, tag="xr")
                nc.sync.dma_start(out=yr[:, :, :], in_=yrows[:, :].rearrange("p (rb d) -> p rb d", d=D))
                nc.sync.dma_start(out=xr[:, :, :], in_=xrows[:, :].rearrange("p (rb d) -> p rb d", d=D))

                # Moving operand: rows 0-63 = xT, rows 64-127 = xT^2.
                # f32: DMA straight in, square in place. bf16/f32r: stage f32
                # and DVE-convert (DMA cast is disabled in this toolchain);
                # the reduced-precision rounding is element-random and washes
                # out in the 67M-term means; diagonal terms stay exact fp32.
                mmdt = {"f32": f32, "bf16": bf16, "f32r": f32r}[MMDT]
                rp_cks = []
                for ck in range(CK):
                    rp = io.tile([128, CHUNK], mmdt, tag=f"rp{ck}")
                    cs = slice(ck * CHUNK, (ck + 1) * CHUNK)
                    if MMDT == "f32":
                        nc.sync.dma_start(out=rp[0:64, :], in_=xT[:, cs])
                        nc.sync.dma_start(out=rp[64:128, :], in_=xT[:, cs])
                        nc.vector.tensor_mul(rp[64:128, :], rp[64:128, :], rp[64:128, :])
                    else:
                        xst = io.tile([128, CHUNK], f32, tag="xst")
                        nc.sync.dma_start(out=xst[0:64, :], in_=xT[:, cs])
                        nc.sync.dma_start(out=xst[64:128, :], in_=xT[:, cs])
                        nc.vector.tensor_copy(rp[0:64, :], xst[0:64, :])
                        nc.vector.tensor_mul(rp[64:128, :], xst[64:128, :], xst[64:128, :])
                    rp_cks.append(rp)

                # Stationary operand per row-block: rows 0-63 = -2*yT_rb, rows 64-127 = 1.
                nc.vector.tensor_scalar_mul(wsb[0:64, :], ytmp[:, :], -2.0)
                if MMDT == "f32r":
                    # Memset can't encode an f32r immediate; cast 1.0s from f32.
                    wscr = io.tile([128, ROWS], f32, tag="wscr")
                    nc.vector.memset(wscr[64:128, :], 1.0)
                    nc.vector.tensor_copy(wsb[64:128, :], wscr[64:128, :])
                else:
                    nc.vector.memset(wsb[64:128, :], 1.0)

                # bias[:, rb] = 1 + sum_d y^2
                bias = setup.tile([128, RB], f32, tag="bias")
                sq_scr = setup.tile([128, RB, D], f32, tag="sq_scr")
                y2t = setup.tile([128, RB], f32, tag="y2t")
                nc.vector.tensor_mul(sq_scr[:, :, :], yr[:, :, :], yr[:, :, :])
                nc.vector.tensor_reduce(
                    out=y2t[:, :], in_=sq_scr[:, :, :], axis=mybir.AxisListType.X, op=OP.add
                )
                nc.vector.tensor_scalar_add(bias[:, :], y2t[:, :], 1.0)

                # Exact diagonal: d2ii = sum_d (y-x)^2 per row.
                diff = setup.tile([128, RB, D], f32, tag="diff")
                nc.vector.tensor_sub(diff[:, :, :], yr[:, :, :], xr[:, :, :])
                sqd = setup.tile([128, RB, D], f32, tag="sqd")
                nc.vector.tensor_mul(sqd[:, :, :], diff[:, :, :], diff[:, :, :])
                d2ii = setup.tile([128, RB], f32, tag="d2ii")
                nc.vector.tensor_reduce(out=d2ii[:, :], in_=sqd[:, :, :], axis=mybir.AxisListType.X, op=OP.add)

                # Per-core output partials: SLr, SRr, SCr, P1, P3, P5, corr3.
                osb = setup.tile([128, OUTW], f32, tag="osb")

                # Diagonal terms via ACT only (reciprocal/ttr are not supported
                # by this runtime): ln(1+d2), r_ii = exp(-ln(1+d2)),
                # s_ii = exp(-ln(2+d2)).
                lnpos = setup.tile([128, RB], f32, tag="lnpos")
                nc.scalar.activation(
                    lnpos[:, :], d2ii[:, :], AF.Ln, bias=1.0, scale=1.0, accum_out=osb[:, 3:4]
                )
                rhat = setup.tile([128, RB], f32, tag="rhat")
                nc.scalar.activation(rhat[:, :], lnpos[:, :], AF.Exp, scale=-1.0)
                d2p2 = setup.tile([128, RB], f32, tag="d2p2")
                nc.vector.tensor_scalar_add(d2p2[:, :], d2ii[:, :], 2.0)
                ln2t = setup.tile([128, RB], f32, tag="ln2t")
                nc.scalar.activation(ln2t[:, :], d2p2[:, :], AF.Ln)
                shat = setup.tile([128, RB], f32, tag="shat")
                nc.scalar.activation(shat[:, :], ln2t[:, :], AF.Exp, scale=-1.0, accum_out=osb[:, 4:5])

                # Main slab: 8 row-blocks x 2 column chunks of [128, 4096].
                accL = setup.tile([128, NCOLS], f32, tag="accL")
                accR = setup.tile([128, NCOLS], f32, tag="accR")
                accC = setup.tile([128, NCOLS], f32, tag="accC")
                for rb in range(RB):
                    if ABLATE == "dma":
                        break
                    w_ap = wsb[:, rb * 128 : (rb + 1) * 128]
                    for ck in range(CK):
                        col = rb * CK + ck
                        v = psum.tile([128, CHUNK], f32, tag="v")
                        for j in range(CHUNK // 512):
                            nc.tensor.matmul(
                                out=v[:, j * 512 : (j + 1) * 512],
                                lhsT=w_ap,
                                rhs=rp_cks[ck][:, j * 512 : (j + 1) * 512],
                                start=True,
                                stop=True,
                            )
                        if ABLATE == "mm":
                            continue
                        L = work.tile([128, CHUNK], f32, tag="L")
                        nc.scalar.activation(
                            L[:, :], v[:, :], AF.Ln,
                            bias=bias[:, rb : rb + 1], scale=1.0,
                            accum_out=accL[:, col : col + 1],
                        )
                        if ABLATE in ("full", "no_stt"):
                            r = work.tile([128, CHUNK], f32, tag="r")
                            nc.scalar.activation(
                                r[:, :], L[:, :], AF.Exp, scale=-1.0,
                                accum_out=accR[:, col : col + 1],
                            )
                        if ABLATE == "full":
                            scr = work.tile([128, CHUNK], f32, tag="scr", bufs=1)
                            nc.vector.scalar_tensor_tensor(
                                out=scr[:, :], in0=r[:, :], scalar=1.0, in1=r[:, :],
                                op0=OP.subtract, op1=OP.mult,
                                accum_out=accC[:, col : col + 1],
                            )

                if ABLATE != "full":
                    if ABLATE in ("mm", "dma"):
                        nc.vector.memset(accL[:, :], 0.0)
                    if ABLATE != "no_stt":
                        nc.vector.memset(accR[:, :], 0.0)
                    nc.vector.memset(accC[:, :], 0.0)
                # Per-row sums over the ck chunks: R (sum r) and C (sum r^2 - r).
                Rall = setup.tile([128, RB], f32, tag="Rall")
                nc.vector.tensor_reduce(
                    out=Rall[:, :],
                    in_=accR[:, :].rearrange("p (rb ck) -> p rb ck", ck=CK),
                    axis=mybir.AxisListType.X,
                    op=OP.add,
                )
                Crow = setup.tile([128, RB], f32, tag="Crow")
                nc.vector.tensor_reduce(
                    out=Crow[:, :],
                    in_=accC[:, :].rearrange("p (rb ck) -> p rb ck", ck=CK),
                    axis=mybir.AxisListType.X,
                    op=OP.add,
                )

                # Per-row logsumexp term: ln(sum_j r - r_ii), accumulated to P5.
                Roff = setup.tile([128, RB], f32, tag="Roff")
                nc.vector.tensor_sub(Roff[:, :], Rall[:, :], rhat[:, :])
                lnr_t = setup.tile([128, RB], f32, tag="lnr_t")
                nc.scalar.activation(lnr_t[:, :], Roff[:, :], AF.Ln, accum_out=osb[:, 5:6])

                # Moment estimate of the dropped sum_j r^3 term, per row:
                # Q = sum r^2, R = sum r; sum r^3 ~= Q^2/R = exp(2 ln Q - ln R).
                Qrow = setup.tile([128, RB], f32, tag="Qrow")
                nc.vector.tensor_add(Qrow[:, :], Rall[:, :], Crow[:, :])
                lnQ = setup.tile([128, RB], f32, tag="lnQ")
                nc.scalar.activation(lnQ[:, :], Qrow[:, :], AF.Ln)
                lnRf = setup.tile([128, RB], f32, tag="lnRf")
                nc.scalar.activation(lnRf[:, :], Rall[:, :], AF.Ln)
                e2 = setup.tile([128, RB], f32, tag="e2")
                nc.vector.scalar_tensor_tensor(
                    out=e2[:, :], in0=lnQ[:, :], scalar=2.0, in1=lnRf[:, :],
                    op0=OP.mult, op1=OP.subtract,
                )
                c3t = setup.tile([128, RB], f32, tag="c3t")
                nc.scalar.activation(c3t[:, :], e2[:, :], AF.Exp, accum_out=osb[:, 6:7])

                # Column-reduce the big accumulators to one column each.
                nc.vector.tensor_reduce(
                    out=osb[:, 0:1],
                    in_=accL[:, :].rearrange("p (a b) -> p a b", a=1),
                    axis=mybir.AxisListType.X,
                    op=OP.add,
                )
                nc.vector.tensor_reduce(
                    out=osb[:, 1:2],
                    in_=accR[:, :].rearrange("p (a b) -> p a b", a=1),
                    axis=mybir.AxisListType.X,
                    op=OP.add,
                )
                nc.vector.tensor_reduce(
                    out=osb[:, 2:3],
                    in_=accC[:, :].rearrange("p (a b) -> p a b", a=1),
                    axis=mybir.AxisListType.X,
                    op=OP.add,
                )

                nc.sync.dma_start(out=o_out[:, :], in_=osb[:, :])

    nc.finalize()
    return nc


def _make_runner(reps=1):
    """Cached jitted shard_map runner over the 8 cores (the multi-core branch
    of bass2jax.run_bass_via_pjrt, kept so repeat calls don't re-jit)."""
    if reps in _RUNNERS:
        return _RUNNERS[reps]
    import jax
    import numpy as _np
    from jax.sharding import Mesh, PartitionSpec
    from jax.experimental.shard_map import shard_map
    import concourse.mybir as mybir
    from concourse import bass2jax

    if reps not in _PROGRAMS:
        _PROGRAMS[reps] = _build_program(reps)
    nc = _PROGRAMS[reps]
    bass2jax.install_neuronx_cc_hook()

    partition_name = nc.partition_id_tensor.name if nc.partition_id_tensor else None
    in_names, out_names, out_avals, zero_shapes = [], [], [], []
    for alloc in nc.m.functions[0].allocations:
        if not isinstance(alloc, mybir.MemoryLocationSet):
            continue
        name = alloc.memorylocations[0].name
        if alloc.kind == "ExternalInput":
            if name != partition_name:
                in_names.append(name)
        elif alloc.kind == "ExternalOutput":
            out_names.append(name)
            shape = tuple(alloc.tensor_shape)
            dtype = mybir.dt.np(alloc.dtype)
            out_avals.append(jax.core.ShapedArray(shape, dtype))
            zero_shapes.append((shape, dtype))
    n_params = len(in_names)
    n_outs = len(out_avals)
    all_names = in_names + out_names
    if partition_name is not None:
        all_names = all_names + [partition_name]
    donate = tuple(range(n_params, n_params + n_outs))

    def _body(*args):
        operands = list(args)
        if partition_name is not None:
            operands.append(bass2jax.partition_id_tensor())
        outs = bass2jax._bass_exec_p.bind(
            *operands,
            out_avals=tuple(out_avals),
            in_names=tuple(all_names),
            out_names=tuple(out_names),
            lowering_input_output_aliases=(),
            sim_require_finite=True,
            sim_require_nnan=True,
            nc=nc,
        )
        return tuple(outs)

    devices = jax.devices()[:NCORES]
    mesh = Mesh(_np.asarray(devices), ("core",))
    in_specs = (PartitionSpec("core"),) * (n_params + n_outs)
    out_specs = (PartitionSpec("core"),) * n_outs
    sharded = jax.jit(
        shard_map(_body, mesh=mesh, in_specs=in_specs, out_specs=out_specs, check_rep=False),
        donate_argnums=donate,
        keep_unused=True,
    )
    _RUNNERS[reps] = (sharded, in_names, out_names, out_avals, zero_shapes)
    return _RUNNERS[reps]


def _prepare_concat_inputs(z_x, z_y):
    """Shard + lay out host inputs (concat of per-core input sets along axis 0),
    then put them on device once with the core sharding so repeat executions
    don't re-pay the host->device transfer."""
    import jax
    import numpy as _np
    from jax.sharding import Mesh, PartitionSpec, NamedSharding

    xT = np.ascontiguousarray(z_x.T)
    per_core = []
    for c in range(NCORES):
        ys = z_y[c * ROWS : (c + 1) * ROWS]
        xs = z_x[c * ROWS : (c + 1) * ROWS]
        per_core.append(
            {
                "xT": xT,
                "yT": np.ascontiguousarray(ys.T),
                "yrows": np.ascontiguousarray(
                    ys.reshape(RB, 128, D).transpose(1, 0, 2).reshape(128, RB * D)
                ),
                "xrows": np.ascontiguousarray(
                    xs.reshape(RB, 128, D).transpose(1, 0, 2).reshape(128, RB * D)
                ),
            }
        )
    _, in_names, _, _, _ = _make_runner(1)
    concat = [
        np.concatenate([per_core[c][name] for c in range(NCORES)], axis=0)
        for name in in_names
    ]
    devices = jax.devices()[:NCORES]
    mesh = Mesh(_np.asarray(devices), ("core",))
    sh = NamedSharding(mesh, PartitionSpec("core"))
    dev = [jax.device_put(a, sh) for a in concat]
    for a in dev:
        a.block_until_ready()
    return dev


def _execute(concat_in, reps=1, fetch=True):
    """Run the cached executable; returns per-core results dicts (fetch=True)
    or the on-device output arrays (fetch=False, for timing)."""
    sharded, in_names, out_names, out_avals, zero_shapes = _make_runner(reps)
    zeros = [np.zeros((NCORES * s[0], *s[1:]), dt) for (s, dt) in zero_shapes]
    out_arrs = sharded(*concat_in, *zeros)
    if not fetch:
        return out_arrs
    return [
        {
            name: np.asarray(out_arrs[i]).reshape(NCORES, *out_avals[i].shape)[c]
            for i, name in enumerate(out_names)
        }
        for c in range(NCORES)
    ]


def kernel(z_x, z_y):
    z_x = np.asarray(z_x, dtype=np.float32)
    z_y = np.asarray(z_y, dtype=np.float32)
    assert z_x.shape == (N, D) and z_y.shape == (N, D)

    results = _execute(_prepare_concat_inputs(z_x, z_y))

    # Host combine (float64): the unshard/all-reduce of per-core scalar partials.
    SL = SC = P1 = P3 = P5 = corr3 = 0.0
    for c in range(NCORES):
        o = results[c]["o_out"].astype(np.float64)
        SL += o[:, 0].sum()
        SC += o[:, 2].sum()
        P1 += o[:, 3].sum()
        P3 += o[:, 4].sum()
        P5 += o[:, 5].sum()
        corr3 += o[:, 6].sum()

    n = float(N)
    mean_pos = -P1 / n
    mean_neg = -(SL - P1) / (n * (n - 1))
    mean_sig_pos = P3 / n
    # sum sigmoid over full slab: sum r - sum r^2 + sum r^3(est); SC = sum(r^2 - r)
    S_sig_all = -SC + corr3
    mean_sig_neg = (S_sig_all - P3) / (n * (n - 1))
    log_baseline = 0.0
    loss = P1 / n + P5 / n - np.log(n - 1)

    return (
        np.float32(mean_pos),
        np.float32(mean_neg),
        np.float32(mean_sig_pos),
        np.float32(mean_sig_neg),
        np.float32(log_baseline),
        np.float32(loss),
    )


# revision 17
# speedup vs baseline: 1.9946x; 1.0182x over previous
"""Trainium2 Bass kernel for nn_DensityRatioEstimator (InfoNCE-style Cauchy-kernel loss).

Math: logits[i,j] = -log(1 + ||z_y_i - z_x_j||^2). All six outputs are scalar
reductions of the 8192x8192 logit matrix. Key identities used on device:
    exp(logit)     = 1/(1+d2)  = r      (logsumexp needs no max-subtraction: r <= 1)
    sigmoid(logit) = 1/(2+d2)  = r/(1+r) ~= r - r^2 + r^3 ...
Per core the slab work per [128, 4096] chunk is: one K=128 fp32 matmul producing
v = x2_j - 2*y_i.x_j (filling all 8 PSUM banks), ACT pass Ln(v + (1+y2_i)) with
fused row-accumulate, ACT pass Exp(-L)=r with fused row-accumulate, and one DVE
scalar_tensor_tensor (r-1)*r with fused row-accumulate. The kernel is ACT-bound:
2 transcendental passes over 8M elems/core at 1 elem/lane/cycle @ 1.2 GHz.
Diagonal terms are recomputed exactly from row-major shards; all per-core
reductions (including the r^3 moment correction sum exp(2*ln Q - ln R)) finish
on device so each core only ships a [128, 7] partial tile to the host, which
combines the 8 cores in float64.

Sharding: rows of z_y across 8 cores (1024 rows each), z_x replicated.

_build_program(reps=K) unrolls the whole body K times inside one NEFF so the
test harness can measure the marginal on-device execution time of one kernel
run, independent of the axon dispatch round-trip.
"""

import os
import numpy as np

N, D = 8192, 64
NCORES = 8
ROWS = N // NCORES          # 1024 z_y rows per core
RB = ROWS // 128            # 8 row-blocks of 128 rows
MMDT = os.environ.get("KERNEL_MMDT", "f32")  # f32 | bf16 | f32r
ABLATE = os.environ.get("KERNEL_ABLATE", "full")  # full | mm_ln | no_stt | mm | dma
# 2048-column chunks with double-buffered PSUM: cross-engine semaphore hops
# measure ~us on this hardware, so the MM->Ln chain must be pipelined two
# chunks deep rather than serialized on a single full-PSUM tile.
CHUNK = int(os.environ.get("KERNEL_CHUNK", "2048"))
CK = N // CHUNK             # column chunks (Ln granularity, PSUM-limited)
NCOLS = RB * CK             # Ln accumulator columns per core
WIDE = 4096                 # Exp/stt granularity (SBUF-sourced, fewer instrs)
SUB = WIDE // CHUNK         # Ln sub-chunks per wide group
NW = N // WIDE              # wide groups per row-block
NWCOLS = RB * NW            # Exp/stt accumulator columns per core
OUTW = 7                    # SLr, SRr, SCr, P1, P3, P5, corr3

_PROGRAMS = {}
_RUNNERS = {}


def _patched_insert_act_table_loads(self):
    """Replace bacc's table-load pass: the stock pass picks a set per
    activation greedily, which with alternating Ln/Exp inserts a ~1.3us
    ACT_TABLE_LOAD before nearly every activation (46us/exec here). All our
    activations are served by the single natural_log_exp_and_others set, so
    one load per block suffices."""
    import concourse.mybir as mybir
    from concourse.hw_specs import get_activation_tables

    tables = list(get_activation_tables(self.m.arch).items())
    combined = next(
        i
        for i, (_nm, fns) in enumerate(tables)
        if mybir.ActivationFunctionType.Ln in fns
        and mybir.ActivationFunctionType.Exp in fns
    )
    fns_comb = tables[combined][1]
    for blk in self.main_func.blocks:
        for idx, inst in enumerate(blk.instructions):
            if isinstance(inst, mybir.InstActivation):
                assert inst.func in fns_comb, inst.func
    for blk in self.main_func.blocks:
        for idx, inst in enumerate(blk.instructions):
            if isinstance(inst, mybir.InstActivation):
                load = mybir.InstLoadActFuncSet(
                    name=self.get_next_instruction_name(),
                    ins=[],
                    outs=[],
                    act_func_set_id=combined,
                )
                load.engine = mybir.EngineType.Activation
                self.register_instruction(load)
                blk.instructions.insert(idx, load)
                break


def _build_program(reps=1):
    import types

    import concourse.bacc as bacc
    import concourse.mybir as mybir
    import concourse.tile as tile

    f32 = mybir.dt.float32
    f32r = mybir.dt.float32r  # noqa: F841 (kept for reference)
    bf16 = mybir.dt.bfloat16
    AF = mybir.ActivationFunctionType
    OP = mybir.AluOpType

    # Bacc (not plain Bass): its compile() pass pipeline splits multi-sem waits
    # (generate_event_semaphores) — required for fp32 self-loading matmuls whose
    # S3_LW struct takes a single wait — and inserts ACT table loads.
    nc = bacc.Bacc("TRN2", target_bir_lowering=False, debug=False)
    nc.insert_act_table_loads = types.MethodType(_patched_insert_act_table_loads, nc)

    xT = nc.dram_tensor("xT", [D, N], f32, kind="ExternalInput")
    yT = nc.dram_tensor("yT", [D, ROWS], f32, kind="ExternalInput")
    yrows = nc.dram_tensor("yrows", [128, RB * D], f32, kind="ExternalInput")
    xrows = nc.dram_tensor("xrows", [128, RB * D], f32, kind="ExternalInput")
    o_out = nc.dram_tensor("o_out", [128, OUTW], f32, kind="ExternalOutput")

    with tile.TileContext(nc) as tc:
        with (
            # bufs=2 so rep r+1's input loads / accumulator writes don't
            # serialize against rep r's tail readers (pipelined back-to-back
            # executions; also overlaps the one-shot startup with nothing).
            tc.tile_pool(name="io", bufs=2) as io,
            tc.tile_pool(name="setup", bufs=2) as setup,
            tc.tile_pool(name="work", bufs=2) as work,
            tc.tile_pool(name="psum", bufs=(8192 // CHUNK) // 2, space="PSUM") as psum,
        ):
            for _rep in range(reps):
                # Small inputs first (one serialized SP DMA queue): they
                # unblock the stationary operand and the bias chain while the
                # big xT chunk loads stream behind them.
                wsb = io.tile([128, ROWS], {"f32": f32, "bf16": bf16, "f32r": f32r}[MMDT], tag="wsb")
                ytmp = io.tile([64, ROWS], f32, tag="ytmp")
                nc.sync.dma_start(out=ytmp[:, :], in_=yT[:, :])

                # Row-major shards for y2 bias + exact diagonal terms.
                yr = io.tile([128, RB, D], f32, tag="yr")
                xr = io.tile([128, RB, D], f32, tag="xr")
                nc.sync.dma_start(out=yr[:, :, :], in_=yrows[:, :].rearrange("p (rb d) -> p rb d", d=D))
                nc.sync.dma_start(out=xr[:, :, :], in_=xrows[:, :].rearrange("p (rb d) -> p rb d", d=D))

                # Moving operand: rows 0-63 = xT, rows 64-127 = xT^2.
                # f32: DMA straight in, square in place. bf16/f32r: stage f32
                # and DVE-convert (DMA cast is disabled in this toolchain);
                # the reduced-precision rounding is element-random and washes
                # out in the 67M-term means; diagonal terms stay exact fp32.
                mmdt = {"f32": f32, "bf16": bf16, "f32r": f32r}[MMDT]
                rp_cks = []
                for ck in range(CK):
                    rp = io.tile([128, CHUNK], mmdt, tag=f"rp{ck}")
                    cs = slice(ck * CHUNK, (ck + 1) * CHUNK)
                    if MMDT == "f32":
                        nc.sync.dma_start(out=rp[0:64, :], in_=xT[:, cs])
                        nc.sync.dma_start(out=rp[64:128, :], in_=xT[:, cs])
                        nc.vector.tensor_mul(rp[64:128, :], rp[64:128, :], rp[64:128, :])
                    else:
                        xst = io.tile([128, CHUNK], f32, tag="xst")
                        nc.sync.dma_start(out=xst[0:64, :], in_=xT[:, cs])
                        nc.sync.dma_start(out=xst[64:128, :], in_=xT[:, cs])
                        nc.vector.tensor_copy(rp[0:64, :], xst[0:64, :])
                        nc.vector.tensor_mul(rp[64:128, :], xst[64:128, :], xst[64:128, :])
                    rp_cks.append(rp)

                # Stationary operand per row-block: rows 0-63 = -2*yT_rb, rows 64-127 = 1.
                nc.vector.tensor_scalar_mul(wsb[0:64, :], ytmp[:, :], -2.0)
                if MMDT == "f32r":
                    # Memset can't encode an f32r immediate; cast 1.0s from f32.
                    wscr = io.tile([128, ROWS], f32, tag="wscr")
                    nc.vector.memset(wscr[64:128, :], 1.0)
                    nc.vector.tensor_copy(wsb[64:128, :], wscr[64:128, :])
                else:
                    nc.vector.memset(wsb[64:128, :], 1.0)

                # bias[:, rb] = 1 + sum_d y^2
                bias = setup.tile([128, RB], f32, tag="bias")
                sq_scr = setup.tile([128, RB, D], f32, tag="sq_scr")
                y2t = setup.tile([128, RB], f32, tag="y2t")
                nc.vector.tensor_mul(sq_scr[:, :, :], yr[:, :, :], yr[:, :, :])
                nc.vector.tensor_reduce(
                    out=y2t[:, :], in_=sq_scr[:, :, :], axis=mybir.AxisListType.X, op=OP.add
                )
                nc.vector.tensor_scalar_add(bias[:, :], y2t[:, :], 1.0)

                # Exact diagonal: d2ii = sum_d (y-x)^2 per row.
                diff = setup.tile([128, RB, D], f32, tag="diff")
                nc.vector.tensor_sub(diff[:, :, :], yr[:, :, :], xr[:, :, :])
                sqd = setup.tile([128, RB, D], f32, tag="sqd")
                nc.vector.tensor_mul(sqd[:, :, :], diff[:, :, :], diff[:, :, :])
                d2ii = setup.tile([128, RB], f32, tag="d2ii")
                nc.vector.tensor_reduce(out=d2ii[:, :], in_=sqd[:, :, :], axis=mybir.AxisListType.X, op=OP.add)

                # Per-core output partials: SLr, SRr, SCr, P1, P3, P5, corr3.
                osb = setup.tile([128, OUTW], f32, tag="osb")

                # Diagonal terms via ACT only (reciprocal/ttr are not supported
                # by this runtime): ln(1+d2), r_ii = exp(-ln(1+d2)),
                # s_ii = exp(-ln(2+d2)).
                lnpos = setup.tile([128, RB], f32, tag="lnpos")
                nc.scalar.activation(
                    lnpos[:, :], d2ii[:, :], AF.Ln, bias=1.0, scale=1.0, accum_out=osb[:, 3:4]
                )
                rhat = setup.tile([128, RB], f32, tag="rhat")
                nc.scalar.activation(rhat[:, :], lnpos[:, :], AF.Exp, scale=-1.0)
                d2p2 = setup.tile([128, RB], f32, tag="d2p2")
                nc.vector.tensor_scalar_add(d2p2[:, :], d2ii[:, :], 2.0)
                ln2t = setup.tile([128, RB], f32, tag="ln2t")
                nc.scalar.activation(ln2t[:, :], d2p2[:, :], AF.Ln)
                shat = setup.tile([128, RB], f32, tag="shat")
                nc.scalar.activation(shat[:, :], ln2t[:, :], AF.Exp, scale=-1.0, accum_out=osb[:, 4:5])

                # Main slab. Ln runs per CHUNK (PSUM-sourced, pipelined via
                # psum bufs); Exp and the DVE stt run per WIDE group from
                # SBUF, halving their per-instruction overheads.
                accL = setup.tile([128, NCOLS], f32, tag="accL")
                accR = setup.tile([128, NWCOLS], f32, tag="accR")
                accC = setup.tile([128, NWCOLS], f32, tag="accC")
                for rb in range(RB):
                    if ABLATE == "dma":
                        break
                    w_ap = wsb[:, rb * 128 : (rb + 1) * 128]
                    for w in range(NW):
                        wcol = rb * NW + w
                        Lw = work.tile([128, WIDE], f32, tag="L")
                        for s in range(SUB):
                            ck = w * SUB + s
                            col = rb * CK + ck
                            v = psum.tile([128, CHUNK], f32, tag="v")
                            for j in range(CHUNK // 512):
                                nc.tensor.matmul(
                                    out=v[:, j * 512 : (j + 1) * 512],
                                    lhsT=w_ap,
                                    rhs=rp_cks[ck][:, j * 512 : (j + 1) * 512],
                                    start=True,
                                    stop=True,
                                )
                            if ABLATE == "mm":
                                continue
                            nc.scalar.activation(
                                Lw[:, s * CHUNK : (s + 1) * CHUNK], v[:, :], AF.Ln,
                                bias=bias[:, rb : rb + 1], scale=1.0,
                                accum_out=accL[:, col : col + 1],
                            )
                        if ABLATE in ("full", "no_stt"):
                            r = work.tile([128, WIDE], f32, tag="r")
                            nc.scalar.activation(
                                r[:, :], Lw[:, :], AF.Exp, scale=-1.0,
                                accum_out=accR[:, wcol : wcol + 1],
                            )
                        if ABLATE == "full":
                            scr = work.tile([128, WIDE], bf16, tag="scr", bufs=1)
                            nc.vector.scalar_tensor_tensor(
                                out=scr[:, :], in0=r[:, :], scalar=1.0, in1=r[:, :],
                                op0=OP.subtract, op1=OP.mult,
                                accum_out=accC[:, wcol : wcol + 1],
                            )

                if ABLATE != "full":
                    if ABLATE in ("mm", "dma"):
                        nc.vector.memset(accL[:, :], 0.0)
                    if ABLATE != "no_stt":
                        nc.vector.memset(accR[:, :], 0.0)
                    nc.vector.memset(accC[:, :], 0.0)
                # Per-row sums over the ck chunks: R (sum r) and C (sum r^2 - r).
                Rall = setup.tile([128, RB], f32, tag="Rall")
                nc.vector.tensor_reduce(
                    out=Rall[:, :],
                    in_=accR[:, :].rearrange("p (rb ck) -> p rb ck", ck=NW),
                    axis=mybir.AxisListType.X,
                    op=OP.add,
                )
                Crow = setup.tile([128, RB], f32, tag="Crow")
                nc.vector.tensor_reduce(
                    out=Crow[:, :],
                    in_=accC[:, :].rearrange("p (rb ck) -> p rb ck", ck=NW),
                    axis=mybir.AxisListType.X,
                    op=OP.add,
                )

                # Per-row logsumexp term: ln(sum_j r - r_ii), accumulated to P5.
                Roff = setup.tile([128, RB], f32, tag="Roff")
                nc.vector.tensor_sub(Roff[:, :], Rall[:, :], rhat[:, :])
                lnr_t = setup.tile([128, RB], f32, tag="lnr_t")
                nc.scalar.activation(lnr_t[:, :], Roff[:, :], AF.Ln, accum_out=osb[:, 5:6])

                # Moment estimate of the dropped sum_j r^3 term, per row:
                # Q = sum r^2, R = sum r; sum r^3 ~= Q^2/R = exp(2 ln Q - ln R).
                Qrow = setup.tile([128, RB], f32, tag="Qrow")
                nc.vector.tensor_add(Qrow[:, :], Rall[:, :], Crow[:, :])
                lnQ = setup.tile([128, RB], f32, tag="lnQ")
                nc.scalar.activation(lnQ[:, :], Qrow[:, :], AF.Ln)
                lnRf = setup.tile([128, RB], f32, tag="lnRf")
                nc.scalar.activation(lnRf[:, :], Rall[:, :], AF.Ln)
                e2 = setup.tile([128, RB], f32, tag="e2")
                nc.vector.scalar_tensor_tensor(
                    out=e2[:, :], in0=lnQ[:, :], scalar=2.0, in1=lnRf[:, :],
                    op0=OP.mult, op1=OP.subtract,
                )
                c3t = setup.tile([128, RB], f32, tag="c3t")
                nc.scalar.activation(c3t[:, :], e2[:, :], AF.Exp, accum_out=osb[:, 6:7])

                # Column-reduce the big accumulators to one column each.
                nc.vector.tensor_reduce(
                    out=osb[:, 0:1],
                    in_=accL[:, :].rearrange("p (a b) -> p a b", a=1),
                    axis=mybir.AxisListType.X,
                    op=OP.add,
                )
                nc.vector.tensor_reduce(
                    out=osb[:, 1:2],
                    in_=accR[:, :].rearrange("p (a b) -> p a b", a=1),
                    axis=mybir.AxisListType.X,
                    op=OP.add,
                )
                nc.vector.tensor_reduce(
                    out=osb[:, 2:3],
                    in_=accC[:, :].rearrange("p (a b) -> p a b", a=1),
                    axis=mybir.AxisListType.X,
                    op=OP.add,
                )

                nc.sync.dma_start(out=o_out[:, :], in_=osb[:, :])

    nc.finalize()
    return nc


def _make_runner(reps=1):
    """Cached jitted shard_map runner over the 8 cores (the multi-core branch
    of bass2jax.run_bass_via_pjrt, kept so repeat calls don't re-jit)."""
    if reps in _RUNNERS:
        return _RUNNERS[reps]
    import jax
    import numpy as _np
    from jax.sharding import Mesh, PartitionSpec
    from jax.experimental.shard_map import shard_map
    import concourse.mybir as mybir
    from concourse import bass2jax

    if reps not in _PROGRAMS:
        _PROGRAMS[reps] = _build_program(reps)
    nc = _PROGRAMS[reps]
    bass2jax.install_neuronx_cc_hook()

    partition_name = nc.partition_id_tensor.name if nc.partition_id_tensor else None
    in_names, out_names, out_avals, zero_shapes = [], [], [], []
    for alloc in nc.m.functions[0].allocations:
        if not isinstance(alloc, mybir.MemoryLocationSet):
            continue
        name = alloc.memorylocations[0].name
        if alloc.kind == "ExternalInput":
            if name != partition_name:
                in_names.append(name)
        elif alloc.kind == "ExternalOutput":
            out_names.append(name)
            shape = tuple(alloc.tensor_shape)
            dtype = mybir.dt.np(alloc.dtype)
            out_avals.append(jax.core.ShapedArray(shape, dtype))
            zero_shapes.append((shape, dtype))
    n_params = len(in_names)
    n_outs = len(out_avals)
    all_names = in_names + out_names
    if partition_name is not None:
        all_names = all_names + [partition_name]
    donate = tuple(range(n_params, n_params + n_outs))

    def _body(*args):
        operands = list(args)
        if partition_name is not None:
            operands.append(bass2jax.partition_id_tensor())
        outs = bass2jax._bass_exec_p.bind(
            *operands,
            out_avals=tuple(out_avals),
            in_names=tuple(all_names),
            out_names=tuple(out_names),
            lowering_input_output_aliases=(),
            sim_require_finite=True,
            sim_require_nnan=True,
            nc=nc,
        )
        return tuple(outs)

    devices = jax.devices()[:NCORES]
    mesh = Mesh(_np.asarray(devices), ("core",))
    in_specs = (PartitionSpec("core"),) * (n_params + n_outs)
    out_specs = (PartitionSpec("core"),) * n_outs
    sharded = jax.jit(
        shard_map(_body, mesh=mesh, in_specs=in_specs, out_specs=out_specs, check_rep=False),
        donate_argnums=donate,
        keep_unused=True,
    )
    _RUNNERS[reps] = (sharded, in_names, out_names, out_avals, zero_shapes)
    return _RUNNERS[reps]


def _prepare_concat_inputs(z_x, z_y):
    """Shard + lay out host inputs (concat of per-core input sets along axis 0),
    then put them on device once with the core sharding so repeat executions
    don't re-pay the host->device transfer."""
    import jax
    import numpy as _np
    from jax.sharding import Mesh, PartitionSpec, NamedSharding

    xT = np.ascontiguousarray(z_x.T)
    per_core = []
    for c in range(NCORES):
        ys = z_y[c * ROWS : (c + 1) * ROWS]
        xs = z_x[c * ROWS : (c + 1) * ROWS]
        per_core.append(
            {
                "xT": xT,
                "yT": np.ascontiguousarray(ys.T),
                "yrows": np.ascontiguousarray(
                    ys.reshape(RB, 128, D).transpose(1, 0, 2).reshape(128, RB * D)
                ),
                "xrows": np.ascontiguousarray(
                    xs.reshape(RB, 128, D).transpose(1, 0, 2).reshape(128, RB * D)
                ),
            }
        )
    _, in_names, _, _, _ = _make_runner(1)
    concat = [
        np.concatenate([per_core[c][name] for c in range(NCORES)], axis=0)
        for name in in_names
    ]
    devices = jax.devices()[:NCORES]
    mesh = Mesh(_np.asarray(devices), ("core",))
    sh = NamedSharding(mesh, PartitionSpec("core"))
    dev = [jax.device_put(a, sh) for a in concat]
    for a in dev:
        a.block_until_ready()
    return dev


def _execute(concat_in, reps=1, fetch=True):
    """Run the cached executable; returns per-core results dicts (fetch=True)
    or the on-device output arrays (fetch=False, for timing)."""
    sharded, in_names, out_names, out_avals, zero_shapes = _make_runner(reps)
    zeros = [np.zeros((NCORES * s[0], *s[1:]), dt) for (s, dt) in zero_shapes]
    out_arrs = sharded(*concat_in, *zeros)
    if not fetch:
        return out_arrs
    return [
        {
            name: np.asarray(out_arrs[i]).reshape(NCORES, *out_avals[i].shape)[c]
            for i, name in enumerate(out_names)
        }
        for c in range(NCORES)
    ]


def kernel(z_x, z_y):
    z_x = np.asarray(z_x, dtype=np.float32)
    z_y = np.asarray(z_y, dtype=np.float32)
    assert z_x.shape == (N, D) and z_y.shape == (N, D)

    results = _execute(_prepare_concat_inputs(z_x, z_y))

    # Host combine (float64): the unshard/all-reduce of per-core scalar partials.
    SL = SC = P1 = P3 = P5 = corr3 = 0.0
    for c in range(NCORES):
        o = results[c]["o_out"].astype(np.float64)
        SL += o[:, 0].sum()
        SC += o[:, 2].sum()
        P1 += o[:, 3].sum()
        P3 += o[:, 4].sum()
        P5 += o[:, 5].sum()
        corr3 += o[:, 6].sum()

    n = float(N)
    mean_pos = -P1 / n
    mean_neg = -(SL - P1) / (n * (n - 1))
    mean_sig_pos = P3 / n
    # sum sigmoid over full slab: sum r - sum r^2 + sum r^3(est); SC = sum(r^2 - r)
    S_sig_all = -SC + corr3
    mean_sig_neg = (S_sig_all - P3) / (n * (n - 1))
    log_baseline = 0.0
    loss = P1 / n + P5 / n - np.log(n - 1)

    return (
        np.float32(mean_pos),
        np.float32(mean_neg),
        np.float32(mean_sig_pos),
        np.float32(mean_sig_neg),
        np.float32(log_baseline),
        np.float32(loss),
    )
